# revision 12
# baseline (speedup 1.0000x reference)
"""GCN encoder (GAE-style) Trainium2 Bass kernel for nn_GCNEncoder_12077448036459.

Math (PyG GCNConv with self-loops, symmetric normalization):
    A_hat = D^-1/2 (A0 + I) D^-1/2          (A0 = directed adjacency, dst<-src)
    h  = relu(A_hat @ (x @ W1) + b1)
    mu = A_hat @ (h @ Wmu) + bmu ; logvar = A_hat @ (h @ Wlv) + blv

Device formulation (per NeuronCore, 8-way dst-node sharding):
    A_hat @ (X @ W) == (A_hat @ X) @ W, so each layer is:
        SpMM (gather+segment-sum over edges)  ->  per-window 128x128 GEMM
    Layer 2 shares one SpMM for mu|logvar via W2 = [Wmu | Wlv].
    The symmetric norm is folded into a row pre-scale of the gather table
    (xs = dinv * x, hs = dinv * relu(...)) plus a per-dst post-scale.

    SpMM: edges sorted by (dst-window, src%4); for each superblock (1024
    dsts) 4 dma_gather calls pull bf16 rows (256B) of the table from HBM
    (int16 gather indices force the src%4 split: idx = src>>2 with
    elem_step=512 covers all 100k rows).  Segment-sum runs on the PE as
    one-hot matmuls: S_t[e, j] = (dst_local[e] == j), accumulated into a
    PSUM [128 dst x 8*128 feat] superblock accumulator.  S is built on the
    DVE with a single is_equal over broadcast APs per gather call.

    Between the two layers the hidden table (dinv*relu(layer1)) is
    AllGather'ed across the 8 cores (HBM->HBM collective).

The host side only does index/plan preparation (sorting edges, padding
cells to the cross-core max so all cores share one SPMD program) and the
structural row scaling of x by dinv.
"""

import math
import os
import sys

import numpy as np

# ---------------------------------------------------------------- constants
N = 100_000
E_RAW = 1_600_000
IN = 128
H = 128
Z = 64
NCORES = 8
SHARD = N // NCORES           # 12500 dst rows per core
WIN = 128                     # dsts per PSUM window
SB_NW = 8                     # windows per superblock
R = 4                         # src mod-4 gather classes
TILE = 128                    # edges per scatter matmul
DMA_SCRATCH = 16384           # SWDGE ring bytes/partition (ring = /16 descs)
GCALL = 1024                  # max idxs per dma_gather call

for _p in ("/opt/trn_rl_repo",):
    if _p not in sys.path and os.path.isdir(_p):
        sys.path.insert(0, _p)


def _cdiv(a, b):
    return -(-a // b)


# ---------------------------------------------------------------- host plan
def build_plan(edge_index, n=N, ncores=NCORES, shard=None):
    """Sort/partition edges and build the per-core gather/scatter blobs.

    Returns dict with global capacities (shared program structure) and
    per-core int16 idx blobs / bf16 dst_local blobs.
    """
    import ml_dtypes

    shard = shard if shard is not None else n // ncores
    nw = _cdiv(shard, WIN)                  # windows per core
    nsb = _cdiv(nw, SB_NW)                  # superblocks per core

    loop = np.arange(n, dtype=np.int64)
    src = np.concatenate([edge_index[0].astype(np.int64), loop])
    dst = np.concatenate([edge_index[1].astype(np.int64), loop])
    deg = np.bincount(dst, minlength=n).astype(np.float64)
    dinv = (1.0 / np.sqrt(deg)).astype(np.float32)

    core = dst // shard
    wloc = (dst - core * shard)             # shard-local dst row
    w = wloc // WIN                         # window id within core
    r = src & 3
    # cell = (core, window, r); sort edges by cell
    cell = ((core * nw + w) * R + r)
    order = np.argsort(cell, kind="stable")
    src_s, dst_s, cell_s = src[order], dst[order], cell[order]
    wloc_s = wloc[order]

    ncells = ncores * nw * R
    counts = np.bincount(cell_s, minlength=ncells).reshape(ncores, nw, R)
    # capacity (in tiles) per (window, r) cell = max over cores, >= 1
    cap_t = np.maximum(1, _cdiv(counts, TILE).max(axis=0))      # [nw, R]

    cell_starts = np.zeros(ncells + 1, dtype=np.int64)
    np.cumsum(counts.reshape(-1), out=cell_starts[1:])

    # per (sb, r): number of tiles = sum over windows in sb
    idx_blobs, dl_blobs = [], []
    sb_meta = []   # per sb: list over r of (n_tiles, [cap tiles per window])
    for sb in range(nsb):
        ws = list(range(sb * SB_NW, min((sb + 1) * SB_NW, nw)))
        per_r = []
        for rr in range(R):
            caps = [int(cap_t[wi, rr]) for wi in ws]
            per_r.append((sum(caps), caps))
        sb_meta.append((ws, per_r))

    total_tiles = int(sum(pr[0] for _, per_r in sb_meta for pr in per_r))
    idx_cols = total_tiles * TILE // 16

    for c in range(ncores):
        idxs = np.zeros((total_tiles * TILE,), dtype=np.int16)
        dls = np.full((128, total_tiles), -1.0, dtype=ml_dtypes.bfloat16)
        t0 = 0
        for sb, (ws, per_r) in enumerate(sb_meta):
            for rr in range(R):
                for wi, capt in zip(ws, per_r[rr][1]):
                    cid = ((c * nw + wi) * R + rr)
                    s, e = cell_starts[cid], cell_starts[cid + 1]
                    cnt = e - s
                    assert cnt <= capt * TILE
                    base = t0 * TILE
                    # gather idx = src >> 2 (call rr reads rows ==rr mod 4)
                    idxs[base : base + cnt] = (src_s[s:e] >> 2).astype(np.int16)
                    # pads keep idx 0 (valid row; weighted 0 by dl == -1)
                    col = np.full((capt * TILE,), -1.0, dtype=np.float32)
                    col[:cnt] = (wloc_s[s:e] - wi * WIN).astype(np.float32)
                    dls[:, t0 : t0 + capt] = (
                        col.reshape(capt, TILE).T.astype(ml_dtypes.bfloat16)
                    )
                    t0 += capt
        assert t0 == total_tiles
        # wrap idxs: idx j -> [j%16, j//16], replicated to 8 groups of 16
        wrapped = idxs.reshape(-1, 16).T            # [16, total/16]
        idx_blobs.append(np.tile(wrapped, (8, 1)))  # [128, idx_cols]
        dl_blobs.append(dls)

    return dict(
        nw=nw, nsb=nsb, sb_meta=sb_meta, cap_t=cap_t,
        total_tiles=total_tiles, idx_cols=idx_cols,
        idx_blobs=idx_blobs, dl_blobs=dl_blobs, dinv=dinv, shard=shard, n=n,
    )


# ------------------------------------------------------------ bass program
def build_program(plan, table_rows, debug=False):
    """Build the SPMD Bass program (shared by all cores)."""
    import concourse.bacc as bacc
    import concourse.tile as tile
    from concourse import bass, mybir
    from concourse.bass import AP

    nw, nsb = plan["nw"], plan["nsb"]
    sb_meta = plan["sb_meta"]
    shard = plan["shard"]
    idx_cols = plan["idx_cols"]
    total_tiles = plan["total_tiles"]
    bf16 = mybir.dt.bfloat16
    f32 = mybir.dt.float32
    i16 = mybir.dt.int16

    nc = bacc.Bacc("TRN2", target_bir_lowering=False, debug=debug,
                   num_devices=NCORES,
                   dynamic_dma_scratch_size=DMA_SCRATCH)

    # ---------------- I/O -------------------------------------------------
    xs_d = nc.dram_tensor("xs", [table_rows, IN], bf16, kind="ExternalInput")
    idx_d = nc.dram_tensor("idxs", [128, idx_cols], i16, kind="ExternalInput")
    dl_d = nc.dram_tensor("dls", [128, total_tiles], bf16, kind="ExternalInput")
    w1_d = nc.dram_tensor("w1", [IN, H], bf16, kind="ExternalInput")
    w2_d = nc.dram_tensor("w2", [H, 2 * Z], bf16, kind="ExternalInput")
    b1_d = nc.dram_tensor("b1t", [128, H], f32, kind="ExternalInput")
    b2_d = nc.dram_tensor("b2t", [128, 2 * Z], f32, kind="ExternalInput")
    dv_d = nc.dram_tensor("dinvc", [128, nw], f32, kind="ExternalInput")
    io_d = nc.dram_tensor("iota", [128, 128], bf16, kind="ExternalInput")
    id_d = nc.dram_tensor("ident", [128, 128], bf16, kind="ExternalInput")
    out_d = nc.dram_tensor("out", [shard, 2 * Z], f32, kind="ExternalOutput")

    hs_shard = nc.dram_tensor("hs_shard", [shard, H], bf16)
    hs_full = nc.dram_tensor("hs_full", [NCORES * shard, H], bf16,
                             addr_space="Shared")

    def table_view(dram_t, rows, rr):
        # strided view: row q -> table row 4q+rr, stride 1024B, 256B payload
        nrow = (rows - rr + 3) // 4
        return AP(dram_t, rr * IN, [[4 * IN, nrow], [1, IN]])

    with tile.TileContext(nc) as tc:
        import contextlib
        with contextlib.ExitStack() as ctx:
            const_p = ctx.enter_context(tc.tile_pool(name="const", bufs=1))
            meta_p = ctx.enter_context(tc.tile_pool(name="meta", bufs=1))
            msg_p = ctx.enter_context(tc.tile_pool(name="msgs", bufs=5))
            s_p = ctx.enter_context(tc.tile_pool(name="smat", bufs=5))
            win_p = ctx.enter_context(tc.tile_pool(name="winsb", bufs=3))
            acc_p = ctx.enter_context(
                tc.tile_pool(name="acc", bufs=2, space="PSUM"))
            ps_p = ctx.enter_context(
                tc.tile_pool(name="pspost", bufs=2, space="PSUM"))

            # constants / metadata resident in SBUF
            w1_s = const_p.tile([IN, H], bf16, tag="w1")
            w2_s = const_p.tile([H, 2 * Z], bf16, tag="w2")
            b1_s = const_p.tile([128, H], f32, tag="b1")
            b2_s = const_p.tile([128, 2 * Z], f32, tag="b2")
            dv_s = const_p.tile([128, nw], f32, tag="dv")
            io_s = const_p.tile([128, 128], bf16, tag="iota")
            id_s = const_p.tile([128, 128], bf16, tag="ident")
            idx_s = meta_p.tile([128, idx_cols], i16, tag="idx")
            dl_s = meta_p.tile([128, total_tiles], bf16, tag="dl")
            for dst_t, src_t in ((w1_s, w1_d), (w2_s, w2_d), (b1_s, b1_d),
                                 (b2_s, b2_d), (dv_s, dv_d), (io_s, io_d),
                                 (id_s, id_d), (idx_s, idx_d), (dl_s, dl_d)):
                nc.sync.dma_start(out=dst_t[:], in_=src_t[:, :])

            def run_phase(phase):
                if phase == 1:
                    tviews = [table_view(xs_d, plan["n"], rr) for rr in range(R)]
                else:
                    tviews = [table_view(hs_full, NCORES * shard, rr)
                              for rr in range(R)]
                w_s = w1_s if phase == 1 else w2_s
                b_s = b1_s if phase == 1 else b2_s
                zdim = H if phase == 1 else 2 * Z

                icol = [0]   # running idx-blob column offset (units of 16)
                tcol = [0]   # running dst-local blob tile offset
                for sb in range(nsb):
                    ws, per_r = sb_meta[sb]
                    acc = acc_p.tile([128, SB_NW * 128], f32, tag="acc")
                    msgs, smats = [], []
                    for rr in range(R):
                        ntiles = per_r[rr][0]
                        nidx = ntiles * TILE
                        m = msg_p.tile([128, ntiles, IN], bf16, tag="m")
                        # split into <= GCALL-idx chunks (SWDGE ring limit)
                        for g0 in range(0, ntiles, GCALL // TILE):
                            gt = min(GCALL // TILE, ntiles - g0)
                            gidx = gt * TILE
                            nc.gpsimd.dma_gather(
                                m[:, g0 : g0 + gt, :], tviews[rr],
                                idx_s[:, icol[0] : icol[0] + gidx // 16],
                                gidx, gidx, IN, elem_step=4 * IN,
                            )
                            icol[0] += gidx // 16
                        # S[e, t, j] = (dst_local[e, t] == iota[j])
                        smat = s_p.tile([128, ntiles * 128], bf16, tag="s")
                        dl_ap = dl_s[:, tcol[0] + sum(
                            pr[0] for pr in per_r[:rr]) :]
                        dl_bc = AP(dl_ap.tensor, dl_ap.offset,
                                   [dl_ap.ap[0], [1, ntiles], [0, 128]])
                        io_ap = io_s[:, :]
                        io_bc = AP(io_ap.tensor, io_ap.offset,
                                   [io_ap.ap[0], [0, ntiles], [1, 128]])
                        s_ap = smat[:]
                        s_3d = AP(s_ap.tensor, s_ap.offset,
                                  [s_ap.ap[0], [128, ntiles], [1, 128]])
                        nc.vector.tensor_tensor(
                            out=s_3d, in0=dl_bc, in1=io_bc,
                            op=mybir.AluOpType.is_equal)
                        msgs.append(m)
                        smats.append(smat)
                    # scatter matmuls: window-major so PSUM accumulation
                    # groups open/close sequentially per window slice
                    for iw, wi in enumerate(ws):
                        for rr in range(R):
                            caps = per_r[rr][1]
                            t_in_r = sum(caps[:iw])
                            for t in range(caps[iw]):
                                nc.tensor.matmul(
                                    out=acc[:, iw * 128 : iw * 128 + 128],
                                    lhsT=smats[rr][
                                        :, (t_in_r + t) * 128 :
                                        (t_in_r + t + 1) * 128],
                                    rhs=msgs[rr][:, t_in_r + t, :],
                                    start=(rr == 0 and t == 0),
                                    stop=(rr == R - 1 and t == caps[iw] - 1),
                                )
                    tcol[0] += sum(pr[0] for pr in per_r)

                    # per-window epilogue
                    for iw, wi in enumerate(ws):
                        rows = min(WIN, shard - wi * WIN)
                        pw = win_p.tile([128, 128], bf16, tag="pw")
                        nc.vector.tensor_copy(
                            out=pw[:], in_=acc[:, iw * 128 : iw * 128 + 128])
                        pt_ps = ps_p.tile([128, 128], bf16, tag="tp")
                        nc.tensor.transpose(
                            out=pt_ps[:], in_=pw[:], identity=id_s[:])
                        pt = win_p.tile([128, 128], bf16, tag="pt")
                        nc.vector.tensor_copy(out=pt[:], in_=pt_ps[:])
                        g_ps = ps_p.tile([128, zdim], f32, tag="gm")
                        nc.tensor.matmul(out=g_ps[:], lhsT=pt[:],
                                         rhs=w_s[:, :], start=True, stop=True)
                        t1 = win_p.tile([128, zdim], f32, tag="t1")
                        nc.vector.tensor_scalar_mul(
                            t1[:], g_ps[:], dv_s[:, wi : wi + 1])
                        if phase == 1:
                            t2 = win_p.tile([128, zdim], f32, tag="t2")
                            nc.vector.tensor_tensor(
                                out=t2[:], in0=t1[:], in1=b_s[:],
                                op=mybir.AluOpType.add)
                            hw = win_p.tile([128, zdim], bf16, tag="hw")
                            # relu(dinv * z) == dinv * relu(z) since dinv > 0
                            nc.scalar.activation(
                                hw[:], t2[:],
                                mybir.ActivationFunctionType.Relu,
                                scale=dv_s[:, wi : wi + 1])
                            nc.sync.dma_start(
                                out=hs_shard[wi * WIN : wi * WIN + rows, :],
                                in_=hw[:rows, :])
                        else:
                            t2 = win_p.tile([128, zdim], f32, tag="t2o")
                            nc.vector.tensor_tensor(
                                out=t2[:], in0=t1[:], in1=b_s[:],
                                op=mybir.AluOpType.add)
                            nc.sync.dma_start(
                                out=out_d[wi * WIN : wi * WIN + rows, :],
                                in_=t2[:rows, :])

            run_phase(1)
            nc.gpsimd.collective_compute(
                "AllGather",
                mybir.AluOpType.bypass,
                replica_groups=[list(range(NCORES))],
                ins=[hs_shard[:, :]],
                outs=[hs_full[:, :]],
            )
            run_phase(2)

    nc.compile()
    return nc


# ---------------------------------------------------------------- staging
def make_in_maps(plan, x, W1, b1, Wmu, bmu, Wlv, blv):
    import ml_dtypes

    bf16 = ml_dtypes.bfloat16
    dinv = plan["dinv"]
    shard, nw = plan["shard"], plan["nw"]
    xs = (np.asarray(x, np.float32) * dinv[:, None]).astype(bf16)
    w1 = np.asarray(W1, np.float32).astype(bf16)
    w2 = np.concatenate([np.asarray(Wmu, np.float32),
                         np.asarray(Wlv, np.float32)], axis=1).astype(bf16)
    b1t = np.tile(np.asarray(b1, np.float32)[None, :], (128, 1))
    b2t = np.tile(np.concatenate([np.asarray(bmu, np.float32),
                                  np.asarray(blv, np.float32)])[None, :],
                  (128, 1))
    iota = np.tile(np.arange(128, dtype=np.float32)[None, :],
                   (128, 1)).astype(bf16)
    ident = np.eye(128, dtype=np.float32).astype(bf16)

    in_maps = []
    for c in range(NCORES):
        dvc = np.zeros((128, nw), np.float32)
        rows = np.arange(shard)
        dvc[rows % WIN, rows // WIN] = dinv[c * shard + rows]
        in_maps.append({
            "xs": xs, "idxs": plan["idx_blobs"][c], "dls": plan["dl_blobs"][c],
            "w1": w1, "w2": w2, "b1t": b1t, "b2t": b2t, "dinvc": dvc,
            "iota": iota, "ident": ident,
        })
    return in_maps


# ------------------------------------------------------------------ kernel
_CACHE = {}


def kernel(x, edge_index, W1, b1, Wmu, bmu, Wlv, blv, trace=False):
    from concourse.bass_utils import run_bass_kernel_spmd

    edge_index = np.asarray(edge_index)
    plan = build_plan(edge_index)
    if "nc" not in _CACHE:
        _CACHE["nc"] = build_program(plan, N)
    nc = _CACHE["nc"]
    in_maps = make_in_maps(plan, x, W1, b1, Wmu, bmu, Wlv, blv)
    res = run_bass_kernel_spmd(nc, in_maps, list(range(NCORES)), trace=trace)
    _CACHE["last_result"] = res
    out = np.concatenate([res.results[c]["out"] for c in range(NCORES)],
                         axis=0).astype(np.float32)
    return (out[:, :Z].copy(), out[:, Z:].copy())


# revision 16
# speedup vs baseline: 2.6586x; 2.6586x over previous
"""GCN encoder (GAE-style) Trainium2 Bass kernel for nn_GCNEncoder_12077448036459.

Math (PyG GCNConv with self-loops, symmetric normalization):
    A_hat = D^-1/2 (A0 + I) D^-1/2          (A0 = directed adjacency, dst<-src)
    h  = relu(A_hat @ (x @ W1) + b1)
    mu = A_hat @ (h @ Wmu) + bmu ; logvar = A_hat @ (h @ Wlv) + blv

Device formulation (per NeuronCore, 8-way dst-node sharding):
    A_hat @ (X @ W) == (A_hat @ X) @ W, so each layer is:
        SpMM (gather+segment-sum over edges)  ->  per-window 128x128 GEMM
    Layer 2 shares one SpMM for mu|logvar via W2 = [Wmu | Wlv].
    The symmetric norm is folded into a row pre-scale of the gather table
    (xs = dinv * x, hs = dinv * relu(...)) plus a per-dst post-scale.

    SpMM: edges sorted by (dst-window, src%4); for each superblock (1024
    dsts) 4 dma_gather calls pull bf16 rows (256B) of the table from HBM
    (int16 gather indices force the src%4 split: idx = src>>2 with
    elem_step=512 covers all 100k rows).  Segment-sum runs on the PE as
    one-hot matmuls: S_t[e, j] = (dst_local[e] == j), accumulated into a
    PSUM [128 dst x 8*128 feat] superblock accumulator.  S is built on the
    DVE with a single is_equal over broadcast APs per gather call.

    Between the two layers the hidden table (dinv*relu(layer1)) is
    AllGather'ed across the 8 cores (HBM->HBM collective).

The host side only does index/plan preparation (sorting edges, padding
cells to the cross-core max so all cores share one SPMD program) and the
structural row scaling of x by dinv.
"""

import math
import os
import sys

import numpy as np

# ---------------------------------------------------------------- constants
N = 100_000
E_RAW = 1_600_000
IN = 128
H = 128
Z = 64
NCORES = 8
SHARD = N // NCORES           # 12500 dst rows per core
WIN = 128                     # dsts per PSUM window
SB_NW = 8                     # windows per superblock
R = 4                         # src mod-4 gather classes
TILE = 128                    # edges per scatter matmul
DMA_SCRATCH = 16384           # SWDGE ring bytes/partition (ring = /16 descs)
GCALL = 1024                  # max idxs per dma_gather call
NQUEUES = 4                   # SWDGE queues (parallel Q7 descriptor gen)

for _p in ("/opt/trn_rl_repo",):
    if _p not in sys.path and os.path.isdir(_p):
        sys.path.insert(0, _p)


def _cdiv(a, b):
    return -(-a // b)


# ---------------------------------------------------------------- host plan
def build_plan(edge_index, n=N, ncores=NCORES, shard=None):
    """Sort/partition edges and build the per-core gather/scatter blobs.

    Returns dict with global capacities (shared program structure) and
    per-core int16 idx blobs / bf16 dst_local blobs.
    """
    import ml_dtypes

    shard = shard if shard is not None else n // ncores
    nw = _cdiv(shard, WIN)                  # windows per core
    nsb = _cdiv(nw, SB_NW)                  # superblocks per core

    loop = np.arange(n, dtype=np.int64)
    src = np.concatenate([edge_index[0].astype(np.int64), loop])
    dst = np.concatenate([edge_index[1].astype(np.int64), loop])
    deg = np.bincount(dst, minlength=n).astype(np.float64)
    dinv = (1.0 / np.sqrt(deg)).astype(np.float32)

    core = dst // shard
    wloc = (dst - core * shard)             # shard-local dst row
    w = wloc // WIN                         # window id within core
    r = src & 3
    # cell = (core, window, r); sort edges by cell
    cell = ((core * nw + w) * R + r)
    order = np.argsort(cell, kind="stable")
    src_s, dst_s, cell_s = src[order], dst[order], cell[order]
    wloc_s = wloc[order]

    ncells = ncores * nw * R
    counts = np.bincount(cell_s, minlength=ncells).reshape(ncores, nw, R)
    # capacity (in tiles) per (window, r) cell = max over cores, >= 1
    cap_t = np.maximum(1, _cdiv(counts, TILE).max(axis=0))      # [nw, R]

    cell_starts = np.zeros(ncells + 1, dtype=np.int64)
    np.cumsum(counts.reshape(-1), out=cell_starts[1:])

    # per (sb, r): number of tiles = sum over windows in sb
    idx_blobs, dl_blobs = [], []
    sb_meta = []   # per sb: list over r of (n_tiles, [cap tiles per window])
    for sb in range(nsb):
        ws = list(range(sb * SB_NW, min((sb + 1) * SB_NW, nw)))
        per_r = []
        for rr in range(R):
            caps = [int(cap_t[wi, rr]) for wi in ws]
            per_r.append((sum(caps), caps))
        sb_meta.append((ws, per_r))

    total_tiles = int(sum(pr[0] for _, per_r in sb_meta for pr in per_r))
    idx_cols = total_tiles * TILE // 16

    for c in range(ncores):
        idxs = np.zeros((total_tiles * TILE,), dtype=np.int16)
        dls = np.full((128, total_tiles), -1.0, dtype=ml_dtypes.bfloat16)
        t0 = 0
        for sb, (ws, per_r) in enumerate(sb_meta):
            for rr in range(R):
                for wi, capt in zip(ws, per_r[rr][1]):
                    cid = ((c * nw + wi) * R + rr)
                    s, e = cell_starts[cid], cell_starts[cid + 1]
                    cnt = e - s
                    assert cnt <= capt * TILE
                    base = t0 * TILE
                    # gather idx = src >> 2 (call rr reads rows ==rr mod 4)
                    idxs[base : base + cnt] = (src_s[s:e] >> 2).astype(np.int16)
                    # pads keep idx 0 (valid row; weighted 0 by dl == -1)
                    col = np.full((capt * TILE,), -1.0, dtype=np.float32)
                    col[:cnt] = (wloc_s[s:e] - wi * WIN).astype(np.float32)
                    dls[:, t0 : t0 + capt] = (
                        col.reshape(capt, TILE).T.astype(ml_dtypes.bfloat16)
                    )
                    t0 += capt
        assert t0 == total_tiles
        # wrap idxs: idx j -> [j%16, j//16], replicated to 8 groups of 16
        wrapped = idxs.reshape(-1, 16).T            # [16, total/16]
        idx_blobs.append(np.tile(wrapped, (8, 1)))  # [128, idx_cols]
        dl_blobs.append(dls)

    return dict(
        nw=nw, nsb=nsb, sb_meta=sb_meta, cap_t=cap_t,
        total_tiles=total_tiles, idx_cols=idx_cols,
        idx_blobs=idx_blobs, dl_blobs=dl_blobs, dinv=dinv, shard=shard, n=n,
    )


# ------------------------------------------------------------ bass program
def build_program(plan, table_rows, debug=False):
    """Build the SPMD Bass program (shared by all cores)."""
    import concourse.bacc as bacc
    import concourse.tile as tile
    from concourse import bass, mybir
    from concourse.bass import AP

    nw, nsb = plan["nw"], plan["nsb"]
    sb_meta = plan["sb_meta"]
    shard = plan["shard"]
    idx_cols = plan["idx_cols"]
    total_tiles = plan["total_tiles"]
    bf16 = mybir.dt.bfloat16
    f32 = mybir.dt.float32
    i16 = mybir.dt.int16

    nc = bacc.Bacc("TRN2", target_bir_lowering=False, debug=debug,
                   num_devices=NCORES,
                   dynamic_dma_scratch_size=DMA_SCRATCH,
                   num_swdge_queues=NQUEUES)

    # ---------------- I/O -------------------------------------------------
    xs_d = nc.dram_tensor("xs", [table_rows, IN], bf16, kind="ExternalInput")
    idx_d = nc.dram_tensor("idxs", [128, idx_cols], i16, kind="ExternalInput")
    dl_d = nc.dram_tensor("dls", [128, total_tiles], bf16, kind="ExternalInput")
    w1_d = nc.dram_tensor("w1", [IN, H], bf16, kind="ExternalInput")
    w2_d = nc.dram_tensor("w2", [H, 2 * Z], bf16, kind="ExternalInput")
    b1_d = nc.dram_tensor("b1t", [128, H], f32, kind="ExternalInput")
    b2_d = nc.dram_tensor("b2t", [128, 2 * Z], f32, kind="ExternalInput")
    dv_d = nc.dram_tensor("dinvc", [128, nw], f32, kind="ExternalInput")
    io_d = nc.dram_tensor("iota", [128, 128], bf16, kind="ExternalInput")
    id_d = nc.dram_tensor("ident", [128, 128], bf16, kind="ExternalInput")
    out_d = nc.dram_tensor("out", [shard, 2 * Z], f32, kind="ExternalOutput")

    hs_shard = nc.dram_tensor("hs_shard", [shard, H], bf16)
    hs_full = nc.dram_tensor("hs_full", [NCORES * shard, H], bf16,
                             addr_space="Shared")

    def table_view(dram_t, rows, rr):
        # strided view: row q -> table row 4q+rr, stride 1024B, 256B payload
        nrow = (rows - rr + 3) // 4
        return AP(dram_t, rr * IN, [[4 * IN, nrow], [1, IN]])

    with tile.TileContext(nc) as tc:
        import contextlib
        with contextlib.ExitStack() as ctx:
            const_p = ctx.enter_context(tc.tile_pool(name="const", bufs=1))
            meta_p = ctx.enter_context(tc.tile_pool(name="meta", bufs=1))
            msg_p = ctx.enter_context(tc.tile_pool(name="msgs", bufs=5))
            s_p = ctx.enter_context(tc.tile_pool(name="smat", bufs=5))
            win_p = ctx.enter_context(tc.tile_pool(name="winsb", bufs=3))
            acc_p = ctx.enter_context(
                tc.tile_pool(name="acc", bufs=2, space="PSUM"))
            ps_p = ctx.enter_context(
                tc.tile_pool(name="pspost", bufs=2, space="PSUM"))

            # constants / metadata resident in SBUF
            w1_s = const_p.tile([IN, H], bf16, tag="w1")
            w2_s = const_p.tile([H, 2 * Z], bf16, tag="w2")
            b1_s = const_p.tile([128, H], f32, tag="b1")
            b2_s = const_p.tile([128, 2 * Z], f32, tag="b2")
            dv_s = const_p.tile([128, nw], f32, tag="dv")
            io_s = const_p.tile([128, 128], bf16, tag="iota")
            id_s = const_p.tile([128, 128], bf16, tag="ident")
            idx_s = meta_p.tile([128, idx_cols], i16, tag="idx")
            dl_s = meta_p.tile([128, total_tiles], bf16, tag="dl")
            for dst_t, src_t in ((w1_s, w1_d), (w2_s, w2_d), (b1_s, b1_d),
                                 (b2_s, b2_d), (dv_s, dv_d), (io_s, io_d),
                                 (id_s, id_d), (idx_s, idx_d), (dl_s, dl_d)):
                nc.sync.dma_start(out=dst_t[:], in_=src_t[:, :])

            def run_phase(phase):
                if phase == 1:
                    tviews = [table_view(xs_d, plan["n"], rr) for rr in range(R)]
                else:
                    tviews = [table_view(hs_full, NCORES * shard, rr)
                              for rr in range(R)]
                w_s = w1_s if phase == 1 else w2_s
                b_s = b1_s if phase == 1 else b2_s
                zdim = H if phase == 1 else 2 * Z

                icol = [0]   # running idx-blob column offset (units of 16)
                tcol = [0]   # running dst-local blob tile offset
                qturn = [0]  # gather queue rotation counter
                for sb in range(nsb):
                    ws, per_r = sb_meta[sb]
                    acc = acc_p.tile([128, SB_NW * 128], f32, tag="acc")
                    msgs, smats = [], []
                    for rr in range(R):
                        ntiles = per_r[rr][0]
                        nidx = ntiles * TILE
                        m = msg_p.tile([128, ntiles, IN], bf16, tag="m")
                        # split into <= GCALL-idx chunks (SWDGE ring limit);
                        # rotate queues so Q7 desc-gen runs in parallel
                        for g0 in range(0, ntiles, GCALL // TILE):
                            gt = min(GCALL // TILE, ntiles - g0)
                            gidx = gt * TILE
                            nc.gpsimd.dma_gather(
                                m[:, g0 : g0 + gt, :], tviews[rr],
                                idx_s[:, icol[0] : icol[0] + gidx // 16],
                                gidx, gidx, IN, elem_step=4 * IN,
                                queue_num=qturn[0] % NQUEUES,
                            )
                            qturn[0] += 1
                            icol[0] += gidx // 16
                        # S[e, t, j] = (dst_local[e, t] == iota[j])
                        smat = s_p.tile([128, ntiles * 128], bf16, tag="s")
                        dl_ap = dl_s[:, tcol[0] + sum(
                            pr[0] for pr in per_r[:rr]) :]
                        dl_bc = AP(dl_ap.tensor, dl_ap.offset,
                                   [dl_ap.ap[0], [1, ntiles], [0, 128]])
                        io_ap = io_s[:, :]
                        io_bc = AP(io_ap.tensor, io_ap.offset,
                                   [io_ap.ap[0], [0, ntiles], [1, 128]])
                        s_ap = smat[:]
                        s_3d = AP(s_ap.tensor, s_ap.offset,
                                  [s_ap.ap[0], [128, ntiles], [1, 128]])
                        nc.vector.tensor_tensor(
                            out=s_3d, in0=dl_bc, in1=io_bc,
                            op=mybir.AluOpType.is_equal)
                        msgs.append(m)
                        smats.append(smat)
                    # scatter matmuls: window-major so PSUM accumulation
                    # groups open/close sequentially per window slice
                    for iw, wi in enumerate(ws):
                        for rr in range(R):
                            caps = per_r[rr][1]
                            t_in_r = sum(caps[:iw])
                            for t in range(caps[iw]):
                                nc.tensor.matmul(
                                    out=acc[:, iw * 128 : iw * 128 + 128],
                                    lhsT=smats[rr][
                                        :, (t_in_r + t) * 128 :
                                        (t_in_r + t + 1) * 128],
                                    rhs=msgs[rr][:, t_in_r + t, :],
                                    start=(rr == 0 and t == 0),
                                    stop=(rr == R - 1 and t == caps[iw] - 1),
                                )
                    tcol[0] += sum(pr[0] for pr in per_r)

                    # per-window epilogue
                    for iw, wi in enumerate(ws):
                        rows = min(WIN, shard - wi * WIN)
                        pw = win_p.tile([128, 128], bf16, tag="pw")
                        nc.vector.tensor_copy(
                            out=pw[:], in_=acc[:, iw * 128 : iw * 128 + 128])
                        pt_ps = ps_p.tile([128, 128], bf16, tag="tp")
                        nc.tensor.transpose(
                            out=pt_ps[:], in_=pw[:], identity=id_s[:])
                        pt = win_p.tile([128, 128], bf16, tag="pt")
                        nc.vector.tensor_copy(out=pt[:], in_=pt_ps[:])
                        g_ps = ps_p.tile([128, zdim], f32, tag="gm")
                        nc.tensor.matmul(out=g_ps[:], lhsT=pt[:],
                                         rhs=w_s[:, :], start=True, stop=True)
                        t1 = win_p.tile([128, zdim], f32, tag="t1")
                        nc.vector.tensor_scalar_mul(
                            t1[:], g_ps[:], dv_s[:, wi : wi + 1])
                        if phase == 1:
                            t2 = win_p.tile([128, zdim], f32, tag="t2")
                            nc.vector.tensor_tensor(
                                out=t2[:], in0=t1[:], in1=b_s[:],
                                op=mybir.AluOpType.add)
                            hw = win_p.tile([128, zdim], bf16, tag="hw")
                            # relu(dinv * z) == dinv * relu(z) since dinv > 0
                            nc.scalar.activation(
                                hw[:], t2[:],
                                mybir.ActivationFunctionType.Relu,
                                scale=dv_s[:, wi : wi + 1])
                            nc.sync.dma_start(
                                out=hs_shard[wi * WIN : wi * WIN + rows, :],
                                in_=hw[:rows, :])
                        else:
                            t2 = win_p.tile([128, zdim], f32, tag="t2o")
                            nc.vector.tensor_tensor(
                                out=t2[:], in0=t1[:], in1=b_s[:],
                                op=mybir.AluOpType.add)
                            nc.sync.dma_start(
                                out=out_d[wi * WIN : wi * WIN + rows, :],
                                in_=t2[:rows, :])

            run_phase(1)
            nc.gpsimd.collective_compute(
                "AllGather",
                mybir.AluOpType.bypass,
                replica_groups=[list(range(NCORES))],
                ins=[hs_shard[:, :]],
                outs=[hs_full[:, :]],
            )
            run_phase(2)

    nc.compile()
    return nc


# ---------------------------------------------------------------- staging
def make_in_maps(plan, x, W1, b1, Wmu, bmu, Wlv, blv):
    import ml_dtypes

    bf16 = ml_dtypes.bfloat16
    dinv = plan["dinv"]
    shard, nw = plan["shard"], plan["nw"]
    xs = (np.asarray(x, np.float32) * dinv[:, None]).astype(bf16)
    w1 = np.asarray(W1, np.float32).astype(bf16)
    w2 = np.concatenate([np.asarray(Wmu, np.float32),
                         np.asarray(Wlv, np.float32)], axis=1).astype(bf16)
    b1t = np.tile(np.asarray(b1, np.float32)[None, :], (128, 1))
    b2t = np.tile(np.concatenate([np.asarray(bmu, np.float32),
                                  np.asarray(blv, np.float32)])[None, :],
                  (128, 1))
    iota = np.tile(np.arange(128, dtype=np.float32)[None, :],
                   (128, 1)).astype(bf16)
    ident = np.eye(128, dtype=np.float32).astype(bf16)

    in_maps = []
    for c in range(NCORES):
        dvc = np.zeros((128, nw), np.float32)
        rows = np.arange(shard)
        dvc[rows % WIN, rows // WIN] = dinv[c * shard + rows]
        in_maps.append({
            "xs": xs, "idxs": plan["idx_blobs"][c], "dls": plan["dl_blobs"][c],
            "w1": w1, "w2": w2, "b1t": b1t, "b2t": b2t, "dinvc": dvc,
            "iota": iota, "ident": ident,
        })
    return in_maps


# ------------------------------------------------------------------ kernel
_CACHE = {}


def kernel(x, edge_index, W1, b1, Wmu, bmu, Wlv, blv, trace=False):
    from concourse.bass_utils import run_bass_kernel_spmd

    edge_index = np.asarray(edge_index)
    plan = build_plan(edge_index)
    if "nc" not in _CACHE:
        _CACHE["nc"] = build_program(plan, N)
    nc = _CACHE["nc"]
    in_maps = make_in_maps(plan, x, W1, b1, Wmu, bmu, Wlv, blv)
    res = run_bass_kernel_spmd(nc, in_maps, list(range(NCORES)), trace=trace)
    _CACHE["last_result"] = res
    out = np.concatenate([res.results[c]["out"] for c in range(NCORES)],
                         axis=0).astype(np.float32)
    return (out[:, :Z].copy(), out[:, Z:].copy())
